# revision 1
# baseline (speedup 1.0000x reference)
"""Multi-head attention TRN2 kernel, head-parallel over 8 NeuronCores.

Reference computation (fp32):
    q,k,v = x@Wq, x@Wk, x@Wv          # [B,S,16*64]
    attn  = softmax(q k^T / 8)         # per head
    out   = (attn @ v) @ Wo            # [B,S,1024]

Sharding: tensor-parallel over heads. Core c owns heads (2c, 2c+1):
Wq/Wk/Wv columns [128c:128c+128], Wo rows [128c:128c+128]. Each core
produces a full-shape partial output; the host sums the 8 partials.

Device-side layout trick: everything is computed in "transposed" space.
The host feeds x^T (D-major, bf16), so projections produce Q^T/K^T
[dh, S] directly (contraction dim D on partitions). Scores are computed
transposed (keys on partitions, queries free), exp'd on ACT without
max-subtraction (|score| <= ~3 for this data distribution, exp is safe
in fp32), and the AV matmul consumes exp-scores directly with contraction
over keys. The softmax denominator comes for free as a 65th column of
ones appended to V (output row 64 of the AV psum = sum_j exp).
"""

from contextlib import ExitStack

import numpy as np

HEADS = 16
DH = 64
D = 1024
B = 4
S = 2048
N_CORES = 8
HPC = HEADS // N_CORES  # heads per core = 2


def build_attention_kernel(nc, b=B, s=S):
    """Emit the per-core program. b/s shrinkable for simulator testing."""
    import concourse.bass as bass
    import concourse.tile as tile
    from concourse import mybir

    bf16 = mybir.dt.bfloat16
    f32 = mybir.dt.float32
    ts = bass.ts

    DC = D // 128          # D chunks of 128 (contraction tiles)
    IC = s // 512          # query chunks of 512 per batch
    JC = s // 128          # key chunks of 128 per batch
    SC = s // 128          # seq chunks of 128 (for V proj / out proj)
    OC = D // 512          # output-dim chunks of 512

    xT_d = nc.dram_tensor("xT", [D, b * s], bf16, kind="ExternalInput").ap()
    # host pre-transposes projection weights to [128, DC, 128] so the
    # load is one contiguous-descriptor DMA instead of a 1024-descriptor
    # gather
    wq_d = nc.dram_tensor("wq", [128, DC, 128], bf16, kind="ExternalInput").ap()
    wk_d = nc.dram_tensor("wk", [128, DC, 128], bf16, kind="ExternalInput").ap()
    wv_d = nc.dram_tensor("wv", [128, DC, 128], bf16, kind="ExternalInput").ap()
    wo_d = nc.dram_tensor("wo", [128, D], bf16, kind="ExternalInput").ap()
    out_d = nc.dram_tensor("out_p", [b * s, D], bf16, kind="ExternalOutput").ap()

    with tile.TileContext(nc) as tc, ExitStack() as ctx:
        wpool = ctx.enter_context(tc.tile_pool(name="weights", bufs=1))
        xpool = ctx.enter_context(tc.tile_pool(name="x", bufs=2))
        qkpool = ctx.enter_context(tc.tile_pool(name="qk", bufs=2))
        vpool = ctx.enter_context(tc.tile_pool(name="v", bufs=2))
        otpool = ctx.enter_context(tc.tile_pool(name="ot", bufs=2))
        expool = ctx.enter_context(tc.tile_pool(name="exp", bufs=2))
        smpool = ctx.enter_context(tc.tile_pool(name="small", bufs=4))
        obpool = ctx.enter_context(tc.tile_pool(name="ob", bufs=5))
        # PSUM budget is 8 banks total:
        #   mm (proj/V/out-proj) 2x[128,512] = 2, scores 2x[128,1024] = 4,
        #   AV 2x[128,512] = 2.
        ps_mm = ctx.enter_context(tc.tile_pool(name="psm", bufs=2, space="PSUM"))
        ps_op = ctx.enter_context(tc.tile_pool(name="psop", bufs=2, space="PSUM"))
        ps_s = ctx.enter_context(tc.tile_pool(name="pss", bufs=2, space="PSUM"))
        ps_o = ctx.enter_context(tc.tile_pool(name="pso", bufs=2, space="PSUM"))

        # --- persistent weights in SBUF, D-chunk major on partitions ---
        wq_sb = wpool.tile([128, DC, 128], bf16, tag="wq")
        wk_sb = wpool.tile([128, DC, 128], bf16, tag="wk")
        wv_sb = wpool.tile([128, DC, 128], bf16, tag="wv")
        wo_sb = wpool.tile([128, D], bf16, tag="wo")
        nc.sync.dma_start(wq_sb[:], wq_d[:])
        nc.sync.dma_start(wk_sb[:], wk_d[:])
        nc.sync.dma_start(wv_sb[:], wv_d[:])
        nc.sync.dma_start(wo_sb[:], wo_d[:])

        # All-ones row used to broadcast the softmax reciprocal across
        # partitions via K=1 outer-product matmuls. bf16 operands keep the
        # col-tiling (tile_position) path ISA-valid; precision is recovered
        # by accumulating a hi + residual pair of outer products in fp32
        # PSUM (error ~1e-5 relative instead of bf16's 4e-3).
        ones64 = wpool.tile([1, 64], bf16, tag="ones64")
        nc.vector.memset(ones64[:], 1.0)

        Exp = mybir.ActivationFunctionType.Exp

        for bi in range(b):
            # --- load x^T slice for this batch: [128, DC, s] bf16 ---
            xb = xpool.tile([128, DC, s], bf16, tag="xb")
            for dc in range(DC):
                nc.sync.dma_start(
                    xb[:, dc, :], xT_d[ts(dc, 128), bi * s : (bi + 1) * s]
                )

            # --- Q^T / K^T projections: [128(2 heads x dh), s] ---
            QT = qkpool.tile([128, s], bf16, tag="qt")
            KT = qkpool.tile([128, s], bf16, tag="kt")
            for w_sb, dst in ((wq_sb, QT), (wk_sb, KT)):
                for ic in range(IC):
                    psq = ps_mm.tile([128, 512], f32, tag="psm")
                    for dc in range(DC):
                        nc.tensor.matmul(
                            psq[:],
                            lhsT=w_sb[:, dc, :],
                            rhs=xb[:, dc, ts(ic, 512)],
                            start=(dc == 0),
                            stop=(dc == DC - 1),
                        )
                    with tc.high_priority():
                        nc.vector.tensor_copy(dst[:, ts(ic, 512)], psq[:])

            # --- V projection, natural (keys-major): [128, SC, 130] ---
            # cols 0:64 = v_h0, col 64 = ones, 65:129 = v_h1, col 129 = ones
            V = vpool.tile([128, SC, 130], bf16, tag="v")
            nc.vector.memset(V[:, :, 64:65], 1.0)
            nc.vector.memset(V[:, :, 129:130], 1.0)
            for sc in range(SC):
                psv = ps_mm.tile([128, 512], f32, tag="psm")
                for dc in range(DC):
                    nc.tensor.matmul(
                        psv[:, 0:128],
                        lhsT=xb[:, dc, ts(sc, 128)],
                        rhs=wv_sb[:, dc, :],
                        start=(dc == 0),
                        stop=(dc == DC - 1),
                    )
                with tc.high_priority():
                    nc.vector.tensor_copy(V[:, sc, 0:64], psv[:, 0:64])
                    nc.vector.tensor_copy(V[:, sc, 65:129], psv[:, 64:128])

            # --- attention, transposed space ---
            # Pipeline skew: emit scores+exp for unit ic, then AV for unit
            # ic-1, so PE always has score matmuls queued while ACT chews
            # through the previous unit's exps.
            OT = otpool.tile([128, s], bf16, tag="ot")

            def emit_scores(ic):
                # ex layout: [128, 2*JC, 512]; slot 2*jc+h holds exp-scores
                # of head h, key-chunk jc, for 512 queries.
                exB = expool.tile([128, 2 * JC, 512], bf16, tag="ex", name="exB")
                for jc in range(JC):
                    for h in range(HPC):
                        hs = h * 64
                        pss = ps_s.tile([128, 512], f32, tag="pss")
                        nc.tensor.matmul(
                            pss[:],
                            lhsT=KT[hs : hs + 64, ts(jc, 128)],
                            rhs=QT[hs : hs + 64, ts(ic, 512)],
                            start=True,
                            stop=True,
                        )
                        nc.scalar.activation(
                            exB[:, 2 * jc + h, :], pss[:], Exp, scale=DH**-0.5
                        )
                return exB

            def emit_av(ic, exB):
                # AV with fused denominator (65th ones column of V).
                psos = []
                for h in range(HPC):
                    pso = ps_o.tile([128, 512], f32, tag="pso")
                    for jc in range(JC):
                        nc.tensor.matmul(
                            pso[0:65, :],
                            lhsT=V[:, jc, h * 65 : h * 65 + 65],
                            rhs=exB[:, 2 * jc + h, :],
                            start=(jc == 0),
                            stop=(jc == JC - 1),
                        )
                    psos.append(pso)
                return psos

            def emit_norm(ic, psos):
                # Deferred normalization: runs ~one unit after its AV so the
                # DVE reciprocal chain never stalls PE's in-order stream.
                for h in range(HPC):
                    pso = psos[h]
                    rc = smpool.tile([1, 512], f32, tag="rc")
                    nc.vector.reciprocal(rc[:], pso[64:65, :])
                    rchi = smpool.tile([1, 512], bf16, tag="rchi")
                    rclo = smpool.tile([1, 512], bf16, tag="rclo")
                    with nc.allow_low_precision(
                        reason="hi+lo bf16 split reassembled in fp32 psum"
                    ):
                        nc.vector.tensor_copy(rchi[:], rc[:])
                        nc.vector.tensor_sub(rclo[:], rc[:], rchi[:])
                    # broadcast 1/denom into pso rows 64:128 (K=1 outer
                    # products with ones; col tile_position targets the
                    # upper partition half), then normalize.
                    nc.tensor.matmul(
                        pso[64:128, :],
                        lhsT=ones64[:],
                        rhs=rchi[:],
                        start=True,
                        stop=False,
                        tile_position=(0, 64),
                    )
                    nc.tensor.matmul(
                        pso[64:128, :],
                        lhsT=ones64[:],
                        rhs=rclo[:],
                        start=False,
                        stop=True,
                        tile_position=(0, 64),
                    )
                    # DVE reads at most one PSUM operand: stage the broadcast
                    # block in SBUF before the normalize multiply.
                    rb = smpool.tile([64, 512], f32, tag="rb")
                    nc.vector.tensor_copy(rb[:], pso[64:128, :])
                    nc.vector.tensor_mul(
                        OT[h * 64 : h * 64 + 64, ts(ic, 512)],
                        pso[0:64, :],
                        rb[:],
                    )

            def emit_outproj(ic):
                # out-proj for the 4 seq-chunks whose OT columns unit ic
                # just normalized; interleaves with the next unit's scores.
                for sc in range(4 * ic, 4 * ic + 4):
                    ob = obpool.tile([128, D], bf16, tag="ob")
                    for oc in range(OC):
                        psp = ps_op.tile([128, 512], f32, tag="psop")
                        nc.tensor.matmul(
                            psp[:],
                            lhsT=OT[:, ts(sc, 128)],
                            rhs=wo_sb[:, ts(oc, 512)],
                            start=True,
                            stop=True,
                        )
                        with tc.high_priority():
                            nc.vector.tensor_copy(ob[:, ts(oc, 512)], psp[:])
                    nc.sync.dma_start(
                        out_d[bi * s + sc * 128 : bi * s + (sc + 1) * 128, :],
                        ob[:],
                    )

            # Steady-state PE order per iteration:
            #   scores(ic) | norm(ic-2) | AV(ic-1)
            # norm(ic-2) must precede AV(ic-1) so the 2-slot pso pool turns
            # over; the scores block between AV and its norm hides the DVE
            # reciprocal latency.
            prev_ex = None
            pending = None
            for ic in range(IC):
                exB = emit_scores(ic)
                if pending is not None:
                    emit_norm(*pending)
                    pending = None
                if prev_ex is not None:
                    psos = emit_av(*prev_ex)
                    pending = (prev_ex[0], psos)
                prev_ex = (ic, exB)
            if pending is not None:
                emit_norm(*pending)
            psos = emit_av(*prev_ex)
            emit_norm(prev_ex[0], psos)
            for ic in range(IC):
                emit_outproj(ic)
    return nc


_NC_CACHE = {}


def _make_nc(b=B, s=S, compile=True):
    from concourse import bacc

    key = (b, s, compile)
    if key in _NC_CACHE:
        return _NC_CACHE[key]
    nc = bacc.Bacc("TRN2", target_bir_lowering=False, debug=False, num_devices=N_CORES)
    build_attention_kernel(nc, b=b, s=s)
    if compile:
        # runs the TRN2 legalization passes (matmul wait splitting, event
        # semaphores, nop fusion) that walrus codegen requires
        nc.compile()
    _NC_CACHE[key] = nc
    return nc


def _wslice(W, sl):
    """[1024, 128] weight slice -> [128, DC, 128] (partition-major chunks)."""
    import ml_dtypes

    w = np.asarray(W, np.float32)[:, sl]
    return np.ascontiguousarray(
        w.reshape(D // 128, 128, 128).transpose(1, 0, 2)
    ).astype(ml_dtypes.bfloat16)


def kernel(x, Wq, Wk, Wv, Wo, _trace=False):
    import ml_dtypes
    from concourse import bass_utils

    bf16 = ml_dtypes.bfloat16
    x = np.asarray(x, dtype=np.float32)
    b, s, d = x.shape
    flat = np.ascontiguousarray(x.reshape(b * s, d))
    xT = np.ascontiguousarray(flat.T).astype(bf16)

    nc = _make_nc(b=b, s=s)

    in_maps = []
    for c in range(N_CORES):
        sl = slice(c * 128, (c + 1) * 128)
        in_maps.append(
            {
                "xT": xT,
                "wq": _wslice(Wq, sl),
                "wk": _wslice(Wk, sl),
                "wv": _wslice(Wv, sl),
                "wo": np.ascontiguousarray(np.asarray(Wo, np.float32)[sl, :]).astype(bf16),
            }
        )

    res = bass_utils.run_bass_kernel_spmd(
        nc, in_maps, core_ids=list(range(N_CORES)), trace=_trace
    )
    acc = np.zeros((b * s, d), np.float32)
    for r in res.results:
        acc += np.asarray(r["out_p"], np.float32)
    out = acc.reshape(b, s, d)
    if _trace:
        kernel._last_results = res
    return out



# revision 18
# speedup vs baseline: 1.3339x; 1.3339x over previous
"""Multi-head attention TRN2 kernel, head-parallel over 8 NeuronCores.

Reference computation (fp32):
    q,k,v = x@Wq, x@Wk, x@Wv          # [B,S,16*64]
    attn  = softmax(q k^T / 8)         # per head
    out   = (attn @ v) @ Wo            # [B,S,1024]

Sharding: tensor-parallel over heads. Core c owns heads (2c, 2c+1):
Wq/Wk/Wv columns [128c:128c+128], Wo rows [128c:128c+128]. Each core
produces a full-shape partial output; the host sums the 8 partials.

v2 design notes (vs the bf16 v1 baseline):
- Projections run as fp8e4m3 DoubleRow matmuls with hi+lo error
  compensation (x = x_hi + x_lo, W = W_hi + W_lo; the lo*lo cross term
  is dropped), giving ~bf16 precision at ~40% of the bf16 PE cost.
- Scores use a q-compensated DoubleRow matmul: lhsT is k8 read through a
  stride-0 pair view, rhs is the (q_hi, q_lo) pair, so s = k8^T(q_hi+q_lo).
  Only the k-side fp8 quantization error survives (~1e-2 of absmax) at
  half the bf16 PE cost.
- exp(s/8) runs on BOTH the ACT engine (native Exp) and the DVE (custom
  8-stage fused op evaluating the minimax polynomial ((a*s+b)^2+c)^16,
  <0.9% rel err over |s/8|<=3.8), splitting the softmax elementwise wall.
  Score PSUM tiles are paired [128,1024] so each exp instruction covers
  two key-chunks (amortizes fixed overheads).
- AV runs "flipped" ([queries, v-dims] output) in bf16: full 128-wide
  partition utilization halves its PE cost vs the v1 orientation, and the
  softmax denominator (ones-column of V) lands per-PARTITION, so the
  normalize is one reciprocal + one tensor_scalar per head-chunk.
- Normalized attention output transposes back to [inner, seq] via PE
  transpose (identity staged from the host) for the bf16 out-proj.
- GPSIMD cannot touch PSUM on TRN2, so all PSUM drains are split across
  ACT (activation-Copy) and DVE; projection work for batch b+1 is
  interleaved into batch b's attention units to keep the PE busy while
  ACT/DVE chew exps.
"""

from contextlib import ExitStack
import dataclasses

import numpy as np

HEADS = 16
DH = 64
D = 1024
B = 4
S = 2048
N_CORES = 8
HPC = HEADS // N_CORES  # heads per core = 2

# minimax fit of ((a*y + b)^2 + c)^16 ~ exp(y) over y in [-3.8, 3.8],
# max rel err 0.89%. y = score/8 is folded into C0 (raw scores in).
_PA = 0.044116274207348274
_PB = 0.7133243299023739
_PC = 0.4912647012807798

# fp8 pre-scales: e4m3 subnormals start at 2^-6, so small-sigma data (W has
# sigma=0.02) must be scaled into the normal range before quantization. The
# scales fold into the PSUM drain copies and the exp() scale.
XS = 8.0    # x pre-scale
WS = 32.0   # W pre-scale
QS = 4.0    # q/k drain re-scale
QK_DRAIN = QS / (XS * WS)          # psq -> q/k fp8
V_DRAIN = 1.0 / (XS * WS)          # psv -> V bf16
SSC = 1.0 / (QS * QS)              # score psum = (1/SSC) * raw score
EXP_SCALE = 0.125 * SSC            # ACT exp scale on score psum
C0R = _PA * EXP_SCALE / 2**0.5 * 2**0.5  # == _PA/8*SSC, kept explicit below
C0R = _PA * 0.125 * SSC

# score key-chunk-PAIR indices whose exp runs on the DVE custom op
DVE_JCPS = frozenset({0, 3, 6})  # of 8 pairs per unit
# engine assignment knobs for the PSUM drains ("act" or "dve")
ENG_V = "dve"
ENG_NORM = "dve"
ENG_TR = "dve"
ENG_OP1 = "dve"
ENG_QLO = "dve"
SCORE_PAIR = True   # pair two key-chunks per PSUM tile / exp instr
PSS_BUFS = 2
PSM_BUFS = 2
HIPRI_EXP = False
SPLIT_PSS = False   # separate score-psum pools for ACT vs DVE exp chains
TR_MODE = "pe"      # "dma" (XBAR) or "pe" (PE transpose + DVE copy)
OP_LAG = 1          # gus between transpose and its out-proj
PSO_QUAD = False    # pack 4 qc AV accumulators into one PSUM bank tile

_EXP_OP = None


def _exp_op():
    """Register (once) the fused DVE op computing the exp polynomial."""
    global _EXP_OP
    if _EXP_OP is not None:
        return _EXP_OP
    from concourse import dve_ops
    from concourse.dve_spec import C0, C1, C2, Spec, Src0, lower, sq
    from concourse.dve_uop import DveOpSpec

    name = "EXP_POLY16_ANT"
    if name in dve_ops._SUB_OPCODE_FOR_NAME:
        _EXP_OP = next(op for op in dve_ops.OPS if op.name == name)
        return _EXP_OP
    w = Src0 * C0 + C1
    base = sq(w) + C2
    body = sq(sq(sq(sq(base))))
    spec = Spec(
        body=body,
        reference=lambda in0, in1, c0, c1, c2: (
            ((in0.astype(np.float32) * c0 + c1) ** 2 + c2) ** 16
        ),
    )
    opcode = max(dve_ops._SUB_OPCODE_FOR_NAME.values()) + 1
    assert opcode < 0x20
    shas = {
        ver: DveOpSpec(
            name=name, opcode=opcode, uops=lower(spec, ver=ver), rd1_en=False
        ).sha(ver)
        for ver in ("v3", "v4")
    }
    dve_ops._SUB_OPCODE_FOR_NAME[name] = opcode
    op = dve_ops.DveOp(name, spec, subdim=False, uops_sha=shas)
    dve_ops.OPS.append(op)
    dve_ops.CUSTOM_DVE_SPECS[name] = spec
    _EXP_OP = op
    return op


def _pairdim(ap, n=2):
    """Insert a stride-0 dim of size n after the partition dim (broadcast)."""
    return dataclasses.replace(ap, ap=[ap.ap[0], [0, n], *ap.ap[1:]])


def _strided2(ap, stride, count, inner):
    """Reshape a [P, F] AP into [P, count, inner] with the given outer stride."""
    return dataclasses.replace(ap, ap=[ap.ap[0], [stride, count], [1, inner]])


def build_attention_kernel(nc, b=B, s=S):
    """Emit the per-core program. b/s shrinkable for simulator testing."""
    import concourse.bass as bass
    import concourse.tile as tile
    from concourse import mybir

    bf16 = mybir.dt.bfloat16
    f8 = mybir.dt.float8e4
    f32 = mybir.dt.float32
    ts = bass.ts
    DR = mybir.MatmulPerfMode.DoubleRow
    Exp = mybir.ActivationFunctionType.Exp
    Copy = mybir.ActivationFunctionType.Copy
    mult = mybir.AluOpType.mult
    subtract = mybir.AluOpType.subtract
    EXP_OP = _exp_op()

    DC = D // 128          # D chunks of 128 (contraction tiles)
    IC = s // 512          # query chunks of 512 per batch
    JC = s // 128          # key chunks of 128 per batch
    JP = JC // 2           # key-chunk pairs
    SC = s // 128          # seq chunks of 128
    OC = D // 512          # output-dim chunks of 512
    NU = 2 * IC            # attention units per batch: (ic, h)

    xhi_d = nc.dram_tensor("xhi", [D, b * s], f8, kind="ExternalInput").ap()
    xlo_d = nc.dram_tensor("xlo", [D, b * s], f8, kind="ExternalInput").ap()
    w_d = {}
    for wn in ("wq", "wk", "wv"):
        for part in ("hi", "lo"):
            w_d[wn, part] = nc.dram_tensor(
                f"{wn}{part}", [128, DC, 128], f8, kind="ExternalInput"
            ).ap()
    wo_d = nc.dram_tensor("wo", [128, D], bf16, kind="ExternalInput").ap()
    id_d = nc.dram_tensor("ident", [128, 128], bf16, kind="ExternalInput").ap()
    out_d = nc.dram_tensor("out_p", [b * s, D], bf16, kind="ExternalOutput").ap()

    with tile.TileContext(nc) as tc, ExitStack() as ctx:
        wpool = ctx.enter_context(tc.tile_pool(name="weights", bufs=1))
        xpool = ctx.enter_context(tc.tile_pool(name="x", bufs=2))
        qkpool = ctx.enter_context(tc.tile_pool(name="qk", bufs=2))
        vpool = ctx.enter_context(tc.tile_pool(name="v", bufs=2))
        otpool = ctx.enter_context(tc.tile_pool(name="ot", bufs=2))
        expool = ctx.enter_context(tc.tile_pool(name="exp", bufs=3))
        anpool = ctx.enter_context(tc.tile_pool(name="an", bufs=2))
        smpool = ctx.enter_context(tc.tile_pool(name="small", bufs=8))
        obpool = ctx.enter_context(tc.tile_pool(name="ob", bufs=4))
        # PSUM: 8 banks total. psm (1 bank x2) is shared by projections,
        # out-proj and the attention transpose; pss holds jc-PAIR score
        # tiles (2 banks x2); pso holds the flipped-AV accumulators.
        ps_mm = ctx.enter_context(tc.tile_pool(name="psm", bufs=PSM_BUFS, space="PSUM"))
        if SPLIT_PSS:
            ps_sA = ctx.enter_context(tc.tile_pool(name="pssA", bufs=1, space="PSUM"))
            ps_sD = ctx.enter_context(tc.tile_pool(name="pssD", bufs=1, space="PSUM"))
        else:
            ps_s = ctx.enter_context(tc.tile_pool(name="pss", bufs=PSS_BUFS, space="PSUM"))
            ps_sA = ps_sD = ps_s
        ps_o = ctx.enter_context(tc.tile_pool(name="pso", bufs=2, space="PSUM"))

        w_sb = {}
        for key, d in w_d.items():
            wt = wpool.tile(
                [128, DC, 128], f8, tag=f"{key[0]}{key[1]}", name=f"{key[0]}{key[1]}"
            )
            w_sb[key] = wt
            nc.sync.dma_start(wt[:], d[:])
        wo_sb = wpool.tile([128, D], bf16, tag="wo")
        nc.sync.dma_start(wo_sb[:], wo_d[:])
        ident = wpool.tile([128, 128], bf16, tag="ident")
        nc.sync.dma_start(ident[:], id_d[:])

        # ---------------- per-batch state handles ----------------
        xb = {}      # bi -> (xhi tile, xlo tile)
        qk = {}      # bi -> (QT8 [128,2,s], K8 [128,s])
        vt = {}      # bi -> V
        ott = {}     # bi -> OT
        exb = {}     # (bi, u) -> exB
        attn = {}    # (bi, ic, qc) -> attn_nat tile
        psos = {}    # (bi, u) -> quad AV psum tile

        def load_x(bi):
            xh = xpool.tile([128, DC, s], f8, tag="xh")
            xl = xpool.tile([128, DC, s], f8, tag="xl")
            for dc in range(DC):
                nc.sync.dma_start(xh[:, dc, :], xhi_d[ts(dc, 128), bi * s : (bi + 1) * s])
                nc.sync.dma_start(xl[:, dc, :], xlo_d[ts(dc, 128), bi * s : (bi + 1) * s])
            xb[bi] = (xh, xl)

        def comp_steps(whi, wlo, xh, xl, cols):
            """DR step list for the compensated projection contraction."""
            steps = []
            for dcp in range(DC // 2):
                dsl = slice(2 * dcp, 2 * dcp + 2)
                steps.append((whi[:, dsl, :], xh[:, dsl, cols]))
                steps.append((wlo[:, dsl, :], xh[:, dsl, cols]))
                steps.append((whi[:, dsl, :], xl[:, dsl, cols]))
            return steps

        def proj_qk(bi, ic):
            """Q and K projection for query-chunk ic of batch bi."""
            xh, xl = xb[bi]
            if ic == 0:
                QT8 = qkpool.tile([128, 2, s], f8, tag="qt")
                K8 = qkpool.tile([128, s], f8, tag="kt")
                qk[bi] = (QT8, K8)
            QT8, K8 = qk[bi]
            for wn in ("wq", "wk"):
                psq = ps_mm.tile([128, 512], f32, tag="psm")
                steps = comp_steps(
                    w_sb[wn, "hi"], w_sb[wn, "lo"], xh, xl, ts(ic, 512)
                )
                n = len(steps)
                for i, (lhsT, rhs) in enumerate(steps):
                    nc.tensor.matmul(
                        psq[:], lhsT=lhsT, rhs=rhs,
                        start=(i == 0), stop=(i == n - 1), perf_mode=DR,
                    )
                with tc.high_priority():
                    if wn == "wq":
                        nc.scalar.activation(
                            QT8[:, 0, ts(ic, 512)], psq[:], Copy, scale=QK_DRAIN
                        )
                        nc.vector.scalar_tensor_tensor(
                            QT8[:, 1, ts(ic, 512)], psq[:], QK_DRAIN,
                            QT8[:, 0, ts(ic, 512)], mult, subtract,
                        )
                    else:
                        nc.scalar.activation(
                            K8[:, ts(ic, 512)], psq[:], Copy, scale=QK_DRAIN
                        )

        def proj_v(bi, sc):
            """V projection for key-chunk sc of batch bi (natural layout)."""
            xh, xl = xb[bi]
            if sc == 0:
                V = vpool.tile([128, SC, 130], bf16, tag="v")
                nc.vector.memset(V[:, :, 64:65], 1.0)
                nc.vector.memset(V[:, :, 129:130], 1.0)
                vt[bi] = V
            V = vt[bi]
            psv = ps_mm.tile([128, 512], f32, tag="psm")
            steps = []
            for dcp in range(DC // 2):
                dsl = slice(2 * dcp, 2 * dcp + 2)
                steps.append((xh[:, dsl, ts(sc, 128)], w_sb["wv", "hi"][:, dsl, :]))
                steps.append((xh[:, dsl, ts(sc, 128)], w_sb["wv", "lo"][:, dsl, :]))
                steps.append((xl[:, dsl, ts(sc, 128)], w_sb["wv", "hi"][:, dsl, :]))
            n = len(steps)
            for i, (lhsT, rhs) in enumerate(steps):
                nc.tensor.matmul(
                    psv[:, 0:128], lhsT=lhsT, rhs=rhs,
                    start=(i == 0), stop=(i == n - 1), perf_mode=DR,
                )
            # one strided scaled copy: psum [128,(2,64)] -> V cols (0:64, 65:129)
            with tc.high_priority():
                if ENG_V == "dve":
                    nc.vector.tensor_scalar(
                        _strided2(V[:, sc, 0:129], 65, 2, 64),
                        _strided2(psv[:, 0:128], 64, 2, 64),
                        V_DRAIN, None, mult,
                    )
                else:
                    nc.scalar.activation(
                        _strided2(V[:, sc, 0:129], 65, 2, 64),
                        _strided2(psv[:, 0:128], 64, 2, 64),
                        Copy, scale=V_DRAIN,
                    )

        def scores_exp_chunk(bi, u, jp):
            """One score-pair (2 key-chunks) + its exp, for unit u=(ic,h)."""
            ic, h = divmod(u, 2)
            QT8, K8 = qk[bi]
            hs = h * 64
            if jp == 0:
                exB = expool.tile([128, JP, 1024], bf16, tag="ex", name=f"ex{bi}_{u}")
                exb[bi, u] = exB
            exB = exb[bi, u]
            from contextlib import nullcontext
            pri = tc.high_priority if HIPRI_EXP else nullcontext
            if SCORE_PAIR:
                dve = jp in DVE_JCPS
                pool = ps_sD if dve else ps_sA
                pss = pool.tile([128, 1024], f32, tag="pssD" if (dve and SPLIT_PSS) else "pss")
                for half in range(2):
                    jc = 2 * jp + half
                    nc.tensor.matmul(
                        pss[:, half * 512:(half + 1) * 512],
                        lhsT=_pairdim(K8[hs:hs + 64, ts(jc, 128)]),
                        rhs=QT8[hs:hs + 64, :, ts(ic, 512)],
                        start=True, stop=True, perf_mode=DR,
                    )
                with pri():
                    if jp in DVE_JCPS:
                        nc.vector._custom_dve(
                            EXP_OP, out=exB[:, jp, :], in0=pss[:],
                            s0=C0R, s1=_PB, imm2=_PC,
                        )
                    else:
                        nc.scalar.activation(
                            exB[:, jp, :], pss[:], Exp, scale=EXP_SCALE
                        )
            else:
                for half in range(2):
                    jc = 2 * jp + half
                    pss = ps_s.tile([128, 512], f32, tag="pss")
                    nc.tensor.matmul(
                        pss[:],
                        lhsT=_pairdim(K8[hs:hs + 64, ts(jc, 128)]),
                        rhs=QT8[hs:hs + 64, :, ts(ic, 512)],
                        start=True, stop=True, perf_mode=DR,
                    )
                    with pri():
                        if jp in DVE_JCPS:
                            nc.vector._custom_dve(
                                EXP_OP,
                                out=exB[:, jp, half * 512:(half + 1) * 512],
                                in0=pss[:], s0=C0R, s1=_PB, imm2=_PC,
                            )
                        else:
                            nc.scalar.activation(
                                exB[:, jp, half * 512:(half + 1) * 512],
                                pss[:], Exp, scale=EXP_SCALE,
                            )

        def av_norm_chunk(bi, u, qc):
            """Flipped AV + normalize for one query sub-chunk of unit u.
            For h==1 chunks, also transposes the finished [q, inner] tile
            into OT."""
            ic, h = divmod(u, 2)
            V = vt[bi]
            exB = exb[bi, u]
            hs = h * 64
            if ic == 0 and h == 0 and qc == 0:
                OT = otpool.tile([128, s], bf16, tag="ot")
                ott[bi] = OT
            if True:
                if PSO_QUAD:
                    if qc == 0:
                        psoq = ps_o.tile([128, 4, 65], f32, tag="pso", name="psoq")
                        psos[bi, u] = psoq
                    psoq = psos[bi, u]
                    if qc == 3:
                        psos.pop((bi, u))
                    pso_out = psoq[:, qc, :]
                    pso_val = psoq[:, qc, 0:64]
                    pso_den = psoq[:, qc, 64:65]
                else:
                    psot = ps_o.tile([128, 65], f32, tag="pso")
                    pso_out = psot[:]
                    pso_val = psot[:, 0:64]
                    pso_den = psot[:, 64:65]
                for jc in range(JC):
                    jp, half = divmod(jc, 2)
                    nc.tensor.matmul(
                        pso_out,
                        lhsT=exB[:, jp, half * 512 + qc * 128: half * 512 + (qc + 1) * 128],
                        rhs=V[:, jc, 65 * h : 65 * h + 65],
                        start=(jc == 0), stop=(jc == JC - 1),
                    )
                if h == 0:
                    an = anpool.tile([128, 128], bf16, tag=f"an{qc}", name=f"an{qc}")
                    attn[bi, ic, qc] = an
                an = attn[bi, ic, qc]
                rc = smpool.tile([128, 1], f32, tag="rc")
                with tc.high_priority():
                    nc.vector.reciprocal(rc[:], pso_den)
                    if ENG_NORM == "dve":
                        nc.vector.tensor_scalar(
                            an[:, hs:hs + 64], pso_val, rc[:, 0:1], None, mult
                        )
                    else:
                        nc.scalar.activation(
                            an[:, hs:hs + 64], pso_val, Copy, scale=rc[:, 0:1]
                        )
            if h == 1:
                if qc == 3:
                    exb.pop((bi, u - 1), None)
                    exb.pop((bi, u), None)
                an = attn.pop((bi, ic, qc))
                OT = ott[bi]
                if TR_MODE == "dma":
                    # XBAR DMA transpose [q, inner] -> [inner, q]; runs on
                    # the (mostly idle) DMA engines instead of PE+PSUM+DVE.
                    nc.sync.dma_start(
                        OT[:, (ic * 4 + qc) * 128:(ic * 4 + qc + 1) * 128],
                        an[:], transpose=True,
                    )
                else:
                    pst = ps_mm.tile([128, 128], bf16, tag="psm", name="pst")
                    nc.tensor.transpose(pst[:], an[:], ident[:])
                    with tc.high_priority():
                        nc.vector.tensor_copy(
                            OT[:, (ic * 4 + qc) * 128:(ic * 4 + qc + 1) * 128],
                            pst[:],
                        )

        def outproj_chunk(bi, ic, i):
            """Out-projection for one seq-chunk of query-chunk ic."""
            OT = ott[bi]
            sc = 4 * ic + i
            ob = obpool.tile([128, D], bf16, tag="ob")
            for oc in range(OC):
                psp = ps_mm.tile([128, 512], f32, tag="psm", name="psp")
                nc.tensor.matmul(
                    psp[:],
                    lhsT=OT[:, ts(sc, 128)],
                    rhs=wo_sb[:, ts(oc, 512)],
                    start=True, stop=True,
                )
                with tc.high_priority():
                    if oc == 0 or ENG_OP1 == "act":
                        nc.scalar.activation(ob[:, ts(oc, 512)], psp[:], Copy)
                    else:
                        nc.vector.tensor_copy(ob[:, ts(oc, 512)], psp[:])
            nc.sync.dma_start(
                out_d[bi * s + sc * 128 : bi * s + (sc + 1) * 128, :], ob[:]
            )

        # ---------------- the global stream ----------------
        # Per global unit gu: weave score-pairs of unit gu with AV chunks
        # of unit gu-2, out-proj chunks of the ic transposed at gu-1, and
        # projection chunks for batch bi+1, so the PE always has non-score
        # work to chew while ACT/DVE drain exps.
        from collections import deque

        load_x(0)
        for ic in range(IC):
            proj_qk(0, ic)
        for sc in range(SC):
            proj_v(0, sc)

        op_ready = deque()
        total_units = b * NU
        for gu in range(total_units + 3):
            bi, u = divmod(gu, NU)
            have_s = gu < total_units
            a_gu = gu - 2
            have_a = 0 <= a_gu < total_units
            abi, au = divmod(max(a_gu, 0), NU)
            ops = []
            while op_ready and op_ready[0][0] <= gu:
                ops.append(op_ready.popleft()[1])

            chunks = []
            if have_s:
                chunks.extend(("s", jp) for jp in range(JP))
            others = []
            if have_a:
                others.extend(("a", qc) for qc in range(4))
            others.extend(("o", oi) for oi in ops)
            weave = []
            si = oi = 0
            for i in range(len(chunks) + len(others)):
                take_other = (i % 2 == 1 and oi < len(others)) or si >= len(chunks)
                if take_other and oi < len(others):
                    weave.append(others[oi]); oi += 1
                else:
                    weave.append(chunks[si]); si += 1

            for kind, arg in weave:
                if kind == "s":
                    scores_exp_chunk(bi, u, arg)
                elif kind == "a":
                    av_norm_chunk(abi, au, arg)
                else:
                    obi, oic, i = arg
                    outproj_chunk(obi, oic, i)

            if have_s and u == 0 and bi + 1 < b:
                load_x(bi + 1)
            if have_a and au % 2 == 1:
                # unit (aic, h1) finished: its 4 seq-chunks are transposed
                aic = au // 2
                for i in range(4):
                    op_ready.append((gu + OP_LAG, (abi, aic, i)))
            # next batch's projection slices
            if have_s and bi + 1 < b:
                if u < IC:
                    proj_qk(bi + 1, u)
                for sc in range(2 * u, 2 * u + 2):
                    proj_v(bi + 1, sc)
        for _, arg in op_ready:
            outproj_chunk(*arg)
    return nc


_NC_CACHE = {}


def _make_nc(b=B, s=S, compile=True):
    from concourse import bacc

    key = (b, s, compile)
    if key in _NC_CACHE:
        return _NC_CACHE[key]
    nc = bacc.Bacc("TRN2", target_bir_lowering=False, debug=False, num_devices=N_CORES)
    build_attention_kernel(nc, b=b, s=s)
    if compile:
        nc.compile()
    _NC_CACHE[key] = nc
    return nc


def _f8(a):
    import ml_dtypes

    return np.asarray(a, np.float32).astype(ml_dtypes.float8_e4m3)


def _wslice_hilo(W, sl):
    """[1024, 128] weight slice -> hi/lo fp8 [128, DC, 128] chunk-major."""
    w = np.asarray(W, np.float32)[:, sl]
    w = np.ascontiguousarray(w.reshape(D // 128, 128, 128).transpose(1, 0, 2)) * WS
    hi = _f8(w)
    lo = _f8(w - hi.astype(np.float32))
    return hi, lo


def kernel(x, Wq, Wk, Wv, Wo, _trace=False):
    import ml_dtypes
    from concourse import bass_utils

    bf16 = ml_dtypes.bfloat16
    x = np.asarray(x, dtype=np.float32)
    b, s, d = x.shape
    flat = np.ascontiguousarray(x.reshape(b * s, d))
    xT = np.ascontiguousarray(flat.T) * XS
    xhi = _f8(xT)
    xlo = _f8(xT - xhi.astype(np.float32))
    ident = np.eye(128, dtype=np.float32).astype(bf16)

    nc = _make_nc(b=b, s=s)

    in_maps = []
    for c in range(N_CORES):
        sl = slice(c * 128, (c + 1) * 128)
        m = {"xhi": xhi, "xlo": xlo, "ident": ident}
        for wn, W in (("wq", Wq), ("wk", Wk), ("wv", Wv)):
            m[f"{wn}hi"], m[f"{wn}lo"] = _wslice_hilo(W, sl)
        m["wo"] = np.ascontiguousarray(np.asarray(Wo, np.float32)[sl, :]).astype(bf16)
        in_maps.append(m)

    res = bass_utils.run_bass_kernel_spmd(
        nc, in_maps, core_ids=list(range(N_CORES)), trace=_trace
    )
    acc = np.zeros((b * s, d), np.float32)
    for r in res.results:
        acc += np.asarray(r["out_p"], np.float32)
    out = acc.reshape(b, s, d)
    if _trace:
        kernel._last_results = res
    return out


# revision 22
# speedup vs baseline: 1.3359x; 1.0015x over previous
"""Multi-head attention TRN2 kernel, head-parallel over 8 NeuronCores.

Reference computation (fp32):
    q,k,v = x@Wq, x@Wk, x@Wv          # [B,S,16*64]
    attn  = softmax(q k^T / 8)         # per head
    out   = (attn @ v) @ Wo            # [B,S,1024]

Sharding: tensor-parallel over heads. Core c owns heads (2c, 2c+1):
Wq/Wk/Wv columns [128c:128c+128], Wo rows [128c:128c+128]. Each core
produces a full-shape partial output; the host sums the 8 partials.

v2 design notes (vs the bf16 v1 baseline):
- Projections run as fp8e4m3 DoubleRow matmuls with hi+lo error
  compensation (x = x_hi + x_lo, W = W_hi + W_lo; the lo*lo cross term
  is dropped), giving ~bf16 precision at ~40% of the bf16 PE cost.
- Scores use a q-compensated DoubleRow matmul: lhsT is k8 read through a
  stride-0 pair view, rhs is the (q_hi, q_lo) pair, so s = k8^T(q_hi+q_lo).
  Only the k-side fp8 quantization error survives (~1e-2 of absmax) at
  half the bf16 PE cost.
- exp(s/8) runs on BOTH the ACT engine (native Exp) and the DVE (custom
  8-stage fused op evaluating the minimax polynomial ((a*s+b)^2+c)^16,
  <0.9% rel err over |s/8|<=3.8), splitting the softmax elementwise wall.
  Score PSUM tiles are paired [128,1024] so each exp instruction covers
  two key-chunks (amortizes fixed overheads).
- AV runs "flipped" ([queries, v-dims] output) in bf16: full 128-wide
  partition utilization halves its PE cost vs the v1 orientation, and the
  softmax denominator (ones-column of V) lands per-PARTITION, so the
  normalize is one reciprocal + one tensor_scalar per head-chunk.
- Normalized attention output transposes back to [inner, seq] via PE
  transpose (identity staged from the host) for the bf16 out-proj.
- GPSIMD cannot touch PSUM on TRN2, so all PSUM drains are split across
  ACT (activation-Copy) and DVE; projection work for batch b+1 is
  interleaved into batch b's attention units to keep the PE busy while
  ACT/DVE chew exps.
"""

from contextlib import ExitStack
import dataclasses

import numpy as np

HEADS = 16
DH = 64
D = 1024
B = 4
S = 2048
N_CORES = 8
HPC = HEADS // N_CORES  # heads per core = 2

# minimax fit of ((a*y + b)^2 + c)^16 ~ exp(y) over y in [-3.8, 3.8],
# max rel err 0.89%. y = score/8 is folded into C0 (raw scores in).
_PA = 0.044116274207348274
_PB = 0.7133243299023739
_PC = 0.4912647012807798

# fp8 pre-scales: e4m3 subnormals start at 2^-6, so small-sigma data (W has
# sigma=0.02) must be scaled into the normal range before quantization. The
# scales fold into the PSUM drain copies and the exp() scale.
XS = 8.0    # x pre-scale
WS = 32.0   # W pre-scale
QS = 4.0    # q/k drain re-scale
QK_DRAIN = QS / (XS * WS)          # psq -> q/k fp8
V_DRAIN = 1.0 / (XS * WS)          # psv -> V bf16
SSC = 1.0 / (QS * QS)              # score psum = (1/SSC) * raw score
EXP_SCALE = 0.125 * SSC            # ACT exp scale on score psum
C0R = _PA * EXP_SCALE / 2**0.5 * 2**0.5  # == _PA/8*SSC, kept explicit below
C0R = _PA * 0.125 * SSC

# score key-chunk-PAIR indices whose exp runs on the DVE custom op
DVE_JCPS = frozenset({0, 3, 6})  # of 8 pairs per unit
# engine assignment knobs for the PSUM drains ("act" or "dve")
ENG_V = "dve"
ENG_NORM = "dve"
ENG_TR = "dve"
ENG_OP1 = "dve"
ENG_QLO = "dve"
SCORE_PAIR = True   # pair two key-chunks per PSUM tile / exp instr
PSS_BUFS = 2
PSM_BUFS = 2
HIPRI_EXP = False
SPLIT_PSS = False   # separate score-psum pools for ACT vs DVE exp chains
TR_MODE = "pe"      # "dma" (XBAR) or "pe" (PE transpose + DVE copy)
OP_LAG = 1          # gus between transpose and its out-proj
PSO_QUAD = False    # pack 4 qc AV accumulators into one PSUM bank tile
EXB_BUFS = 4
AN_BUFS = 2
OB_BUFS = 4

_EXP_OP = None


def _exp_op():
    """Register (once) the fused DVE op computing the exp polynomial."""
    global _EXP_OP
    if _EXP_OP is not None:
        return _EXP_OP
    from concourse import dve_ops
    from concourse.dve_spec import C0, C1, C2, Spec, Src0, lower, sq
    from concourse.dve_uop import DveOpSpec

    name = "EXP_POLY16_ANT"
    if name in dve_ops._SUB_OPCODE_FOR_NAME:
        _EXP_OP = next(op for op in dve_ops.OPS if op.name == name)
        return _EXP_OP
    w = Src0 * C0 + C1
    base = sq(w) + C2
    body = sq(sq(sq(sq(base))))
    spec = Spec(
        body=body,
        reference=lambda in0, in1, c0, c1, c2: (
            ((in0.astype(np.float32) * c0 + c1) ** 2 + c2) ** 16
        ),
    )
    opcode = max(dve_ops._SUB_OPCODE_FOR_NAME.values()) + 1
    assert opcode < 0x20
    shas = {
        ver: DveOpSpec(
            name=name, opcode=opcode, uops=lower(spec, ver=ver), rd1_en=False
        ).sha(ver)
        for ver in ("v3", "v4")
    }
    dve_ops._SUB_OPCODE_FOR_NAME[name] = opcode
    op = dve_ops.DveOp(name, spec, subdim=False, uops_sha=shas)
    dve_ops.OPS.append(op)
    dve_ops.CUSTOM_DVE_SPECS[name] = spec
    _EXP_OP = op
    return op


def _pairdim(ap, n=2):
    """Insert a stride-0 dim of size n after the partition dim (broadcast)."""
    return dataclasses.replace(ap, ap=[ap.ap[0], [0, n], *ap.ap[1:]])


def _strided2(ap, stride, count, inner):
    """Reshape a [P, F] AP into [P, count, inner] with the given outer stride."""
    return dataclasses.replace(ap, ap=[ap.ap[0], [stride, count], [1, inner]])


def build_attention_kernel(nc, b=B, s=S):
    """Emit the per-core program. b/s shrinkable for simulator testing."""
    import concourse.bass as bass
    import concourse.tile as tile
    from concourse import mybir

    bf16 = mybir.dt.bfloat16
    f8 = mybir.dt.float8e4
    f32 = mybir.dt.float32
    ts = bass.ts
    DR = mybir.MatmulPerfMode.DoubleRow
    Exp = mybir.ActivationFunctionType.Exp
    Copy = mybir.ActivationFunctionType.Copy
    mult = mybir.AluOpType.mult
    subtract = mybir.AluOpType.subtract
    EXP_OP = _exp_op()

    DC = D // 128          # D chunks of 128 (contraction tiles)
    IC = s // 512          # query chunks of 512 per batch
    JC = s // 128          # key chunks of 128 per batch
    JP = JC // 2           # key-chunk pairs
    SC = s // 128          # seq chunks of 128
    OC = D // 512          # output-dim chunks of 512
    NU = 2 * IC            # attention units per batch: (ic, h)

    xhi_d = nc.dram_tensor("xhi", [D, b * s], f8, kind="ExternalInput").ap()
    xlo_d = nc.dram_tensor("xlo", [D, b * s], f8, kind="ExternalInput").ap()
    w_d = {}
    for wn in ("wq", "wk", "wv"):
        for part in ("hi", "lo"):
            w_d[wn, part] = nc.dram_tensor(
                f"{wn}{part}", [128, DC, 128], f8, kind="ExternalInput"
            ).ap()
    wo_d = nc.dram_tensor("wo", [128, D], bf16, kind="ExternalInput").ap()
    id_d = nc.dram_tensor("ident", [128, 128], bf16, kind="ExternalInput").ap()
    out_d = nc.dram_tensor("out_p", [b * s, D], bf16, kind="ExternalOutput").ap()

    with tile.TileContext(nc) as tc, ExitStack() as ctx:
        wpool = ctx.enter_context(tc.tile_pool(name="weights", bufs=1))
        xpool = ctx.enter_context(tc.tile_pool(name="x", bufs=2))
        qkpool = ctx.enter_context(tc.tile_pool(name="qk", bufs=2))
        vpool = ctx.enter_context(tc.tile_pool(name="v", bufs=2))
        otpool = ctx.enter_context(tc.tile_pool(name="ot", bufs=2))
        expool = ctx.enter_context(tc.tile_pool(name="exp", bufs=EXB_BUFS))
        anpool = ctx.enter_context(tc.tile_pool(name="an", bufs=AN_BUFS))
        smpool = ctx.enter_context(tc.tile_pool(name="small", bufs=8))
        obpool = ctx.enter_context(tc.tile_pool(name="ob", bufs=OB_BUFS))
        # PSUM: 8 banks total. psm (1 bank x2) is shared by projections,
        # out-proj and the attention transpose; pss holds jc-PAIR score
        # tiles (2 banks x2); pso holds the flipped-AV accumulators.
        ps_mm = ctx.enter_context(tc.tile_pool(name="psm", bufs=PSM_BUFS, space="PSUM"))
        if SPLIT_PSS:
            ps_sA = ctx.enter_context(tc.tile_pool(name="pssA", bufs=1, space="PSUM"))
            ps_sD = ctx.enter_context(tc.tile_pool(name="pssD", bufs=1, space="PSUM"))
        else:
            ps_s = ctx.enter_context(tc.tile_pool(name="pss", bufs=PSS_BUFS, space="PSUM"))
            ps_sA = ps_sD = ps_s
        ps_o = ctx.enter_context(tc.tile_pool(name="pso", bufs=2, space="PSUM"))

        w_sb = {}
        for key, d in w_d.items():
            wt = wpool.tile(
                [128, DC, 128], f8, tag=f"{key[0]}{key[1]}", name=f"{key[0]}{key[1]}"
            )
            w_sb[key] = wt
            nc.sync.dma_start(wt[:], d[:])
        wo_sb = wpool.tile([128, D], bf16, tag="wo")
        nc.sync.dma_start(wo_sb[:], wo_d[:])
        ident = wpool.tile([128, 128], bf16, tag="ident")
        nc.sync.dma_start(ident[:], id_d[:])

        # ---------------- per-batch state handles ----------------
        xb = {}      # bi -> (xhi tile, xlo tile)
        qk = {}      # bi -> (QT8 [128,2,s], K8 [128,s])
        vt = {}      # bi -> V
        ott = {}     # bi -> OT
        exb = {}     # (bi, u) -> exB
        attn = {}    # (bi, ic, qc) -> attn_nat tile
        psos = {}    # (bi, u) -> quad AV psum tile

        def load_x(bi):
            xh = xpool.tile([128, DC, s], f8, tag="xh")
            xl = xpool.tile([128, DC, s], f8, tag="xl")
            for dc in range(DC):
                nc.sync.dma_start(xh[:, dc, :], xhi_d[ts(dc, 128), bi * s : (bi + 1) * s])
                nc.sync.dma_start(xl[:, dc, :], xlo_d[ts(dc, 128), bi * s : (bi + 1) * s])
            xb[bi] = (xh, xl)

        def comp_steps(whi, wlo, xh, xl, cols):
            """DR step list for the compensated projection contraction."""
            steps = []
            for dcp in range(DC // 2):
                dsl = slice(2 * dcp, 2 * dcp + 2)
                steps.append((whi[:, dsl, :], xh[:, dsl, cols]))
                steps.append((wlo[:, dsl, :], xh[:, dsl, cols]))
                steps.append((whi[:, dsl, :], xl[:, dsl, cols]))
            return steps

        def proj_qk(bi, ic):
            """Q and K projection for query-chunk ic of batch bi."""
            xh, xl = xb[bi]
            if ic == 0:
                QT8 = qkpool.tile([128, 2, s], f8, tag="qt")
                K8 = qkpool.tile([128, s], f8, tag="kt")
                qk[bi] = (QT8, K8)
            QT8, K8 = qk[bi]
            for wn in ("wq", "wk"):
                psq = ps_mm.tile([128, 512], f32, tag="psm")
                steps = comp_steps(
                    w_sb[wn, "hi"], w_sb[wn, "lo"], xh, xl, ts(ic, 512)
                )
                n = len(steps)
                for i, (lhsT, rhs) in enumerate(steps):
                    nc.tensor.matmul(
                        psq[:], lhsT=lhsT, rhs=rhs,
                        start=(i == 0), stop=(i == n - 1), perf_mode=DR,
                    )
                with tc.high_priority():
                    if wn == "wq":
                        nc.scalar.activation(
                            QT8[:, 0, ts(ic, 512)], psq[:], Copy, scale=QK_DRAIN
                        )
                        nc.vector.scalar_tensor_tensor(
                            QT8[:, 1, ts(ic, 512)], psq[:], QK_DRAIN,
                            QT8[:, 0, ts(ic, 512)], mult, subtract,
                        )
                    else:
                        nc.scalar.activation(
                            K8[:, ts(ic, 512)], psq[:], Copy, scale=QK_DRAIN
                        )

        def proj_v(bi, sc):
            """V projection for key-chunk sc of batch bi (natural layout)."""
            xh, xl = xb[bi]
            if sc == 0:
                V = vpool.tile([128, SC, 130], bf16, tag="v")
                nc.vector.memset(V[:, :, 64:65], 1.0)
                nc.vector.memset(V[:, :, 129:130], 1.0)
                vt[bi] = V
            V = vt[bi]
            psv = ps_mm.tile([128, 512], f32, tag="psm")
            steps = []
            for dcp in range(DC // 2):
                dsl = slice(2 * dcp, 2 * dcp + 2)
                steps.append((xh[:, dsl, ts(sc, 128)], w_sb["wv", "hi"][:, dsl, :]))
                steps.append((xh[:, dsl, ts(sc, 128)], w_sb["wv", "lo"][:, dsl, :]))
                steps.append((xl[:, dsl, ts(sc, 128)], w_sb["wv", "hi"][:, dsl, :]))
            n = len(steps)
            for i, (lhsT, rhs) in enumerate(steps):
                nc.tensor.matmul(
                    psv[:, 0:128], lhsT=lhsT, rhs=rhs,
                    start=(i == 0), stop=(i == n - 1), perf_mode=DR,
                )
            # one strided scaled copy: psum [128,(2,64)] -> V cols (0:64, 65:129)
            with tc.high_priority():
                if ENG_V == "dve":
                    nc.vector.tensor_scalar(
                        _strided2(V[:, sc, 0:129], 65, 2, 64),
                        _strided2(psv[:, 0:128], 64, 2, 64),
                        V_DRAIN, None, mult,
                    )
                else:
                    nc.scalar.activation(
                        _strided2(V[:, sc, 0:129], 65, 2, 64),
                        _strided2(psv[:, 0:128], 64, 2, 64),
                        Copy, scale=V_DRAIN,
                    )

        def scores_exp_chunk(bi, u, jp):
            """One score-pair (2 key-chunks) + its exp, for unit u=(ic,h)."""
            ic, h = divmod(u, 2)
            QT8, K8 = qk[bi]
            hs = h * 64
            if jp == 0:
                exB = expool.tile([128, JP, 1024], bf16, tag="ex", name=f"ex{bi}_{u}")
                exb[bi, u] = exB
            exB = exb[bi, u]
            from contextlib import nullcontext
            pri = tc.high_priority if HIPRI_EXP else nullcontext
            if SCORE_PAIR:
                dve = jp in DVE_JCPS
                pool = ps_sD if dve else ps_sA
                pss = pool.tile([128, 1024], f32, tag="pssD" if (dve and SPLIT_PSS) else "pss")
                for half in range(2):
                    jc = 2 * jp + half
                    nc.tensor.matmul(
                        pss[:, half * 512:(half + 1) * 512],
                        lhsT=_pairdim(K8[hs:hs + 64, ts(jc, 128)]),
                        rhs=QT8[hs:hs + 64, :, ts(ic, 512)],
                        start=True, stop=True, perf_mode=DR,
                    )
                with pri():
                    if jp in DVE_JCPS:
                        nc.vector._custom_dve(
                            EXP_OP, out=exB[:, jp, :], in0=pss[:],
                            s0=C0R, s1=_PB, imm2=_PC,
                        )
                    else:
                        nc.scalar.activation(
                            exB[:, jp, :], pss[:], Exp, scale=EXP_SCALE
                        )
            else:
                for half in range(2):
                    jc = 2 * jp + half
                    pss = ps_s.tile([128, 512], f32, tag="pss")
                    nc.tensor.matmul(
                        pss[:],
                        lhsT=_pairdim(K8[hs:hs + 64, ts(jc, 128)]),
                        rhs=QT8[hs:hs + 64, :, ts(ic, 512)],
                        start=True, stop=True, perf_mode=DR,
                    )
                    with pri():
                        if jp in DVE_JCPS:
                            nc.vector._custom_dve(
                                EXP_OP,
                                out=exB[:, jp, half * 512:(half + 1) * 512],
                                in0=pss[:], s0=C0R, s1=_PB, imm2=_PC,
                            )
                        else:
                            nc.scalar.activation(
                                exB[:, jp, half * 512:(half + 1) * 512],
                                pss[:], Exp, scale=EXP_SCALE,
                            )

        def av_norm_chunk(bi, u, qc):
            """Flipped AV + normalize for one query sub-chunk of unit u.
            For h==1 chunks, also transposes the finished [q, inner] tile
            into OT."""
            ic, h = divmod(u, 2)
            V = vt[bi]
            exB = exb[bi, u]
            hs = h * 64
            if ic == 0 and h == 0 and qc == 0:
                OT = otpool.tile([128, s], bf16, tag="ot")
                ott[bi] = OT
            if True:
                if PSO_QUAD:
                    if qc == 0:
                        psoq = ps_o.tile([128, 4, 65], f32, tag="pso", name="psoq")
                        psos[bi, u] = psoq
                    psoq = psos[bi, u]
                    if qc == 3:
                        psos.pop((bi, u))
                    pso_out = psoq[:, qc, :]
                    pso_val = psoq[:, qc, 0:64]
                    pso_den = psoq[:, qc, 64:65]
                else:
                    psot = ps_o.tile([128, 65], f32, tag="pso")
                    pso_out = psot[:]
                    pso_val = psot[:, 0:64]
                    pso_den = psot[:, 64:65]
                for jc in range(JC):
                    jp, half = divmod(jc, 2)
                    nc.tensor.matmul(
                        pso_out,
                        lhsT=exB[:, jp, half * 512 + qc * 128: half * 512 + (qc + 1) * 128],
                        rhs=V[:, jc, 65 * h : 65 * h + 65],
                        start=(jc == 0), stop=(jc == JC - 1),
                    )
                if h == 0:
                    an = anpool.tile([128, 128], bf16, tag=f"an{qc}", name=f"an{qc}")
                    attn[bi, ic, qc] = an
                an = attn[bi, ic, qc]
                rc = smpool.tile([128, 1], f32, tag="rc")
                with tc.high_priority():
                    nc.vector.reciprocal(rc[:], pso_den)
                    if ENG_NORM == "dve":
                        nc.vector.tensor_scalar(
                            an[:, hs:hs + 64], pso_val, rc[:, 0:1], None, mult
                        )
                    else:
                        nc.scalar.activation(
                            an[:, hs:hs + 64], pso_val, Copy, scale=rc[:, 0:1]
                        )
            if h == 1:
                if qc == 3:
                    exb.pop((bi, u - 1), None)
                    exb.pop((bi, u), None)
                an = attn.pop((bi, ic, qc))
                OT = ott[bi]
                if TR_MODE == "dma":
                    # XBAR DMA transpose [q, inner] -> [inner, q]; runs on
                    # the (mostly idle) DMA engines instead of PE+PSUM+DVE.
                    nc.sync.dma_start(
                        OT[:, (ic * 4 + qc) * 128:(ic * 4 + qc + 1) * 128],
                        an[:], transpose=True,
                    )
                else:
                    pst = ps_mm.tile([128, 128], bf16, tag="psm", name="pst")
                    nc.tensor.transpose(pst[:], an[:], ident[:])
                    with tc.high_priority():
                        nc.vector.tensor_copy(
                            OT[:, (ic * 4 + qc) * 128:(ic * 4 + qc + 1) * 128],
                            pst[:],
                        )

        def outproj_chunk(bi, ic, i):
            """Out-projection for one seq-chunk of query-chunk ic."""
            OT = ott[bi]
            sc = 4 * ic + i
            ob = obpool.tile([128, D], bf16, tag="ob")
            for oc in range(OC):
                psp = ps_mm.tile([128, 512], f32, tag="psm", name="psp")
                nc.tensor.matmul(
                    psp[:],
                    lhsT=OT[:, ts(sc, 128)],
                    rhs=wo_sb[:, ts(oc, 512)],
                    start=True, stop=True,
                )
                with tc.high_priority():
                    if oc == 0 or ENG_OP1 == "act":
                        nc.scalar.activation(ob[:, ts(oc, 512)], psp[:], Copy)
                    else:
                        nc.vector.tensor_copy(ob[:, ts(oc, 512)], psp[:])
            nc.sync.dma_start(
                out_d[bi * s + sc * 128 : bi * s + (sc + 1) * 128, :], ob[:]
            )

        # ---------------- the global stream ----------------
        # Per global unit gu: weave score-pairs of unit gu with AV chunks
        # of unit gu-2, out-proj chunks of the ic transposed at gu-1, and
        # projection chunks for batch bi+1, so the PE always has non-score
        # work to chew while ACT/DVE drain exps.
        from collections import deque

        load_x(0)
        for ic in range(IC):
            proj_qk(0, ic)
        for sc in range(SC):
            proj_v(0, sc)

        op_ready = deque()
        total_units = b * NU
        for gu in range(total_units + 3):
            bi, u = divmod(gu, NU)
            have_s = gu < total_units
            a_gu = gu - 2
            have_a = 0 <= a_gu < total_units
            abi, au = divmod(max(a_gu, 0), NU)
            ops = []
            while op_ready and op_ready[0][0] <= gu:
                ops.append(op_ready.popleft()[1])

            chunks = []
            if have_s:
                chunks.extend(("s", jp) for jp in range(JP))
            others = []
            if have_a:
                others.extend(("a", qc) for qc in range(4))
            others.extend(("o", oi) for oi in ops)
            weave = []
            si = oi = 0
            for i in range(len(chunks) + len(others)):
                take_other = (i % 2 == 1 and oi < len(others)) or si >= len(chunks)
                if take_other and oi < len(others):
                    weave.append(others[oi]); oi += 1
                else:
                    weave.append(chunks[si]); si += 1

            for kind, arg in weave:
                if kind == "s":
                    scores_exp_chunk(bi, u, arg)
                elif kind == "a":
                    av_norm_chunk(abi, au, arg)
                else:
                    obi, oic, i = arg
                    outproj_chunk(obi, oic, i)

            if have_s and u == 0 and bi + 1 < b:
                load_x(bi + 1)
            if have_a and au % 2 == 1:
                # unit (aic, h1) finished: its 4 seq-chunks are transposed
                aic = au // 2
                for i in range(4):
                    op_ready.append((gu + OP_LAG, (abi, aic, i)))
            # next batch's projection slices
            if have_s and bi + 1 < b:
                if u < IC:
                    proj_qk(bi + 1, u)
                for sc in range(2 * u, 2 * u + 2):
                    proj_v(bi + 1, sc)
        for _, arg in op_ready:
            outproj_chunk(*arg)
    return nc


_NC_CACHE = {}


def _make_nc(b=B, s=S, compile=True):
    from concourse import bacc

    key = (b, s, compile)
    if key in _NC_CACHE:
        return _NC_CACHE[key]
    nc = bacc.Bacc("TRN2", target_bir_lowering=False, debug=False, num_devices=N_CORES)
    build_attention_kernel(nc, b=b, s=s)
    if compile:
        nc.compile()
    _NC_CACHE[key] = nc
    return nc


def _f8(a):
    import ml_dtypes

    return np.asarray(a, np.float32).astype(ml_dtypes.float8_e4m3)


def _wslice_hilo(W, sl):
    """[1024, 128] weight slice -> hi/lo fp8 [128, DC, 128] chunk-major."""
    w = np.asarray(W, np.float32)[:, sl]
    w = np.ascontiguousarray(w.reshape(D // 128, 128, 128).transpose(1, 0, 2)) * WS
    hi = _f8(w)
    lo = _f8(w - hi.astype(np.float32))
    return hi, lo


def kernel(x, Wq, Wk, Wv, Wo, _trace=False):
    import ml_dtypes
    from concourse import bass_utils

    bf16 = ml_dtypes.bfloat16
    x = np.asarray(x, dtype=np.float32)
    b, s, d = x.shape
    flat = np.ascontiguousarray(x.reshape(b * s, d))
    xT = np.ascontiguousarray(flat.T) * XS
    xhi = _f8(xT)
    xlo = _f8(xT - xhi.astype(np.float32))
    ident = np.eye(128, dtype=np.float32).astype(bf16)

    nc = _make_nc(b=b, s=s)

    in_maps = []
    for c in range(N_CORES):
        sl = slice(c * 128, (c + 1) * 128)
        m = {"xhi": xhi, "xlo": xlo, "ident": ident}
        for wn, W in (("wq", Wq), ("wk", Wk), ("wv", Wv)):
            m[f"{wn}hi"], m[f"{wn}lo"] = _wslice_hilo(W, sl)
        m["wo"] = np.ascontiguousarray(np.asarray(Wo, np.float32)[sl, :]).astype(bf16)
        in_maps.append(m)

    res = bass_utils.run_bass_kernel_spmd(
        nc, in_maps, core_ids=list(range(N_CORES)), trace=_trace
    )
    acc = np.zeros((b * s, d), np.float32)
    for r in res.results:
        acc += np.asarray(r["out_p"], np.float32)
    out = acc.reshape(b, s, d)
    if _trace:
        kernel._last_results = res
    return out
